# revision 48
# baseline (speedup 1.0000x reference)
"""COMPASSNet MoE-routing kernel for 8 TRN2 NeuronCores (v3).

Problem: B=262144 samples of D=32 features with NaNs at 0/1/2 positions;
each of P=529 NaN patterns owns a tiny MLP (32 -> 4 -> 1, tanh/sigmoid).
y[b] = sigmoid(W2[p].tanh(x0[b] @ W1[p] + b1[p]) + b2[p]), p = pattern id.

Design: weights are the STATIONARY matmul operand (16-column LDWEIGHTS,
~13ns), X is the MOVING operand streaming at 1 column/cycle.

Host pack: samples sorted by pattern; each pattern split into k near-equal
chunks (k chosen globally so chunk count = 128*NB and sizes are uniform);
chunks dealt round-robin across 8 cores in size order.  Per core: NB banks
of 16 chunks (4 strips x 4 bands); bank b's column width W_b = its largest
chunk (pad columns are zero, ~1.5% total).  Banks group by 4 for the
sigmoid/output stage.

Device per bank b (W = bankW[b]):
  MM1 (x4):  lhsT = strip_j [128, 16] (4 patterns' W1, block diag),
             rhs = X_bj [128, W] (4 bands = 4 chunks' features),
             out = psum1[32j:32j+16, :W] - h on partitions, samples free.
  tanh:      ACT psum1 -> th (f16) with per-partition bias b1 (no carrier
             tricks; pattern 0 runs on device too).
  MM2:       lhsT = w2 block [128, 16], rhs = th,
             out = psum2[32q:32q+16, :W] (q = b%4) - the matmul does the
             cross-h reduction.
  sigmoid:   one ACT per 4-bank group on psum2 with per-partition bias b2
             (the matmul-unwritten psum regions it reads are only ever
             stale-finite because the zeroing matmuls ran first; their
             outputs are discarded at unpack).

Startup: a DVE memset + dummy tanh pull the ~1.3us ACT table load off the
critical path; six zeros x zeros matmuls zero every PSUM bank we use (so
partial-partition matmul writes never meet NaN garbage) and keep the PE
busy across the initial DMA wait.  Weights ride the Scalar HWDGE ring in
parallel with the x stream on Sync (DMA instruction issue costs ~0.7us
each, so few large transfers); MM2 trails MM1 by two banks so the in-order
Tensor queue never stalls on a fresh tanh.  Output order is unscrambled on
the host.
"""

import heapq

import numpy as np

import concourse.bass as bass
import concourse.tile as tile
from concourse import mybir
from concourse.bass_utils import run_bass_kernel_spmd


F32 = mybir.dt.float32
F16 = mybir.dt.float16
NP16 = np.float16

B = 262144
D = 32
P = 529
H = 4
N_CORES = 8
NB = 9          # banks per core (16 chunks each)


def _group_sizes(nb):
    # trailing groups shrink (…,3,2) so the tail-of-kernel sigmoid+y chain
    # covers only the narrowest banks
    gs = []
    while nb > 5:
        gs.append(4)
        nb -= 4
    if nb > 2:
        gs.append(nb - 2)
        nb = 2
    if nb:
        gs.append(nb)
    return gs


# ----------------------------------------------------------------- host pack
def _plan_chunks(counts):
    """Split patterns into 128*NB near-equal chunks.

    Returns chunks [(size, pattern, offset)] sorted by size desc and the
    per-group widths Wg (group g covers banks 4g..; W = its largest chunk).
    """
    target = 128 * NB
    h = []
    npieces = 0
    for p, n in enumerate(counts):
        n = int(n)
        if n == 0:
            continue
        k = (n + 511) // 512  # no piece may exceed 512 (PSUM bank width)
        heapq.heappush(h, (-((n + k - 1) // k), p, k))
        npieces += k
    assert npieces <= target, f"{npieces} chunks > {target} slots"
    while npieces < target:
        _, p, k = heapq.heappop(h)
        n = int(counts[p])
        k += 1
        heapq.heappush(h, (-((n + k - 1) // k), p, k))
        npieces += 1
    kmap = {p: k for _, p, k in h}
    chunks = []
    for p, n in enumerate(counts):
        n = int(n)
        if n == 0:
            continue
        k = kmap[p]
        base, rem = divmod(n, k)
        off = 0
        for i in range(k):
            sz = base + (1 if i < rem else 0)
            chunks.append((sz, p, off))
            off += sz
    chunks.sort(key=lambda c: -c[0])
    assert len(chunks) == target
    gsizes = _group_sizes(NB)
    # per-bank width = its largest chunk (chunks dealt 128 per bank row);
    # group width (sigmoid/y extent) = its first=widest bank
    bankW = [max(1, chunks[b * 128][0]) for b in range(NB)]
    Wg = []
    boff = 0
    for gs in gsizes:
        Wg.append(bankW[boff])
        boff += gs
    return chunks, bankW, Wg, gsizes


def _pack(x, pattern_ids, W1, b1, W2, b2):
    pid = np.asarray(pattern_ids).astype(np.int64).ravel()
    x0 = np.nan_to_num(np.asarray(x, dtype=np.float32))
    W1 = np.asarray(W1, dtype=np.float32)
    b1 = np.asarray(b1, dtype=np.float32)
    W2 = np.asarray(W2, dtype=np.float32)
    b2 = np.asarray(b2, dtype=np.float32)

    order = np.argsort(pid, kind="stable")
    counts = np.bincount(pid, minlength=P)
    starts = np.zeros(P + 1, np.int64)
    np.cumsum(counts, out=starts[1:])

    chunks, bankW, Wg, gsizes = _plan_chunks(counts)
    NG = len(gsizes)
    b2gq = []
    for g, gs in enumerate(gsizes):
        b2gq += [(g, q) for q in range(gs)]
    xoff = np.zeros(NB + 1, np.int64)
    for b in range(NB):
        xoff[b + 1] = xoff[b] + 4 * bankW[b]
    XC = int(xoff[NB])
    yoff = np.zeros(NG + 1, np.int64)
    for g in range(NG):
        yoff[g + 1] = yoff[g] + Wg[g]
    YC = int(yoff[NG])

    # ws layout (f16 cols): strips NB*64 | w2 NB*16 | b1(f32 as 2xf16) NB*2
    # | b2(f32 as 2xf16) NG*2
    W2OFF = NB * 64
    B1OFF = NB * 80
    B2OFF = B1OFF + NB * 2
    WS = B2OFF + NG * 2
    xs = [np.zeros((128, XC), NP16) for _ in range(N_CORES)]
    ws = [np.zeros((128, WS), NP16) for _ in range(N_CORES)]
    b1f = [np.zeros((128, NB), np.float32) for _ in range(N_CORES)]
    b2f = [np.zeros((128, NG), np.float32) for _ in range(N_CORES)]
    scat = [[] for _ in range(N_CORES)]  # (row, ycol0, n, sample_idx_array)

    hh = np.arange(H)
    for rank, (sz, p, off) in enumerate(chunks):
        if sz == 0:
            continue
        c = rank % N_CORES
        pos = rank // N_CORES
        b, k = divmod(pos, 16)
        j, t = divmod(k, 4)
        g, q = b2gq[b]
        W = bankW[b]
        samples = order[starts[p] + off: starts[p] + off + sz]
        xs[c][32 * t:32 * t + 32, xoff[b] + j * W: xoff[b] + j * W + sz] = \
            x0[samples].T
        ws[c][32 * t:32 * t + 32, (b * 4 + j) * 16 + 4 * t + hh] = W1[p]
        ws[c][32 * j + 4 * t + hh, W2OFF + b * 16 + 4 * j + t] = W2[p]
        b1f[c][32 * j + 4 * t + hh, b] = b1[p]
        b2f[c][32 * q + 4 * j + t, g] = b2[p]
        scat[c].append((32 * q + 4 * j + t, int(yoff[g]), sz, samples))

    for c in range(N_CORES):
        ws[c][:, B1OFF:B1OFF + NB * 2] = b1f[c].view(NP16)
        ws[c][:, B2OFF:B2OFF + NG * 2] = b2f[c].view(NP16)

    in_maps = [{"xs": xs[c], "ws": ws[c]} for c in range(N_CORES)]
    return bankW, gsizes, Wg, XC, YC, WS, in_maps, scat


# ------------------------------------------------------------- device build
def _split_excess_waits(nc, cap=1):
    """walrus rejects >1 sync wait per instruction; move extras onto
    same-engine NoOps placed immediately before the owner."""
    f = nc.m.functions[0]
    for bb in list(f.blocks):
        out, changed = [], False
        for inst in bb.instructions:
            si = inst.sync_info
            waits = list(si.on_wait) if si is not None else []
            if len(waits) > cap:
                for w in waits[:-cap]:
                    out.append(mybir.InstNoOp(
                        name=nc.get_next_instruction_name(),
                        sync_info=mybir.SyncInfo(on_wait=[w], on_update=[]),
                        bass_nofuse=True,
                        engine=inst.engine,
                    ))
                si.on_wait = waits[-cap:]
                changed = True
            out.append(inst)
        if changed:
            bb.instructions = out
    return nc


def _build(bankW, gsizes, Wg, XC, YC, WS):
    NG = len(gsizes)
    b2gq = []
    gend = []
    for g, gs in enumerate(gsizes):
        b2gq += [(g, q) for q in range(gs)]
        gend.append(len(b2gq) - 1)
    W2OFF = NB * 64
    B1OFF = NB * 80
    B2OFF = B1OFF + NB * 2
    nc = bass.Bass("TRN2", target_bir_lowering=False, debug=False)
    xs = nc.declare_dram_parameter("xs", [128, XC], F16, isOutput=False)
    wsd = nc.declare_dram_parameter("ws", [128, WS], F16, isOutput=False)
    y = nc.declare_dram_parameter("y", [128, YC], F16, isOutput=True)

    xoff = [0]
    for b in range(NB):
        xoff.append(xoff[-1] + 4 * bankW[b])
    yoff = [0]
    for g in range(NG):
        yoff.append(yoff[-1] + Wg[g])

    with tile.TileContext(nc) as tc:
        with (
            tc.tile_pool(name="consts", bufs=1) as consts,
            tc.tile_pool(name="rot", bufs=1) as rot,
            tc.tile_pool(name="ps", bufs=1, space="PSUM") as psp,
        ):
            # zero f16 source for the zeroing matmuls (DVE memset: Scalar
            # stays free for its weight DMA + ACT table load)
            wu = consts.tile([128, 512], F16)
            nc.vector.memset(wu, 0.0)

            # weights ride the Scalar HWDGE ring, in parallel with the x
            # stream on Sync; the dummy activation right after pulls the
            # ~1.3us ACT_TABLE_LOAD off the first real tanh's critical path
            # (tanh+sigmoid share a table set).  DMA instruction issue costs
            # ~0.7us each, so both streams use few, large transfers.
            ws_sb = consts.tile([128, WS], F16)
            nc.scalar.dma_start(out=ws_sb, in_=wsd[:, :])
            dummy = consts.tile([128, 1], F32)
            nc.scalar.activation(out=dummy, in_=wu[:, :1],
                                 func=mybir.ActivationFunctionType.Tanh)

            # x stream split across BOTH HWDGE rings (Sync + Scalar): total
            # HBM bandwidth is shared, but per-ring completion receipts
            # (~1.5-3us each) overlap instead of serializing, and the two
            # first chunks issue concurrently.  Single-bank chunks at the
            # head (earliest MM1 start) and tail (early tail semaphores).
            xts = {}
            # few large chunks: DMA completion receipts inflate with
            # descriptor-queue depth, so minimizing in-flight transfers gets
            # the tail banks' semaphores to fire sooner.
            # four large chunks: DMA completion receipts inflate with
            # descriptor-queue depth, so in-flight transfer count matters
            # more than per-bank granularity.
            chunk_plan = [
                (nc.sync, 0, 2),
                (nc.sync, 2, 4),
                (nc.sync, 4, 7),
                (nc.sync, 7, 9),
            ]
            assert chunk_plan[-1][2] == NB
            for ci, (eng, b0, b1e) in enumerate(chunk_plan):
                xt = consts.tile([128, xoff[b1e] - xoff[b0]], F16,
                                 tag=f"xt{ci}", name=f"xt{ci}")
                eng.dma_start(out=xt, in_=xs[:, xoff[b0]:xoff[b1e]])
                for b in range(b0, b1e):
                    xts[b] = (xt, xoff[b] - xoff[b0])

            y_sb = consts.tile([128, YC], F16)

            # PSUM zeroing (zeros x zeros): allocate all six tiles up front
            # so pool slots align bank b -> slot b%4, but emit the matmuls
            # spread through the first banks - they fill PE gaps while the
            # x stream ramps, keeping the HAM activity window busy (2.4GHz).
            zt = [psp.tile([128, 512], F32, tag="ps1", name=f"z1_{i}",
                           bufs=4) for i in range(4)]
            zt += [psp.tile([128, 512], F32, tag="ps2", name=f"z2_{i}",
                            bufs=2) for i in range(2)]

            def zero_mm(i):
                nc.tensor.matmul(out=zt[i], lhsT=wu[:, :128], rhs=wu,
                                 start=True, stop=True)

            ps1s, ths, ps2s = {}, {}, {}

            def emit_mm1(b):
                W = bankW[b]
                ps1 = psp.tile([128, 512], F32, tag="ps1", name=f"ps1_{b}",
                               bufs=4)
                ps1s[b] = ps1
                xt, xo = xts[b]
                for j in range(4):
                    nc.tensor.matmul(
                        out=ps1[32 * j:32 * j + 16, :W],
                        lhsT=ws_sb[:, (b * 4 + j) * 16:(b * 4 + j + 1) * 16],
                        rhs=xt[:, xo + j * W: xo + (j + 1) * W],
                        start=True, stop=True,
                        tile_position=(0, 32 * j),
                    )

            def emit_tanh(b):
                W = bankW[b]
                th = rot.tile([128, 512], F16, tag="th", name=f"th_{b}",
                              bufs=5)
                ths[b] = th
                nc.scalar.activation(
                    out=th[:, :W], in_=ps1s[b][:, :W],
                    func=mybir.ActivationFunctionType.Tanh,
                    bias=ws_sb[:, B1OFF + 2 * b:B1OFF + 2 * b + 2].bitcast(F32))

            def emit_mm2(b):
                W = bankW[b]
                g, q = b2gq[b]
                if q == 0:
                    ps2s[g] = psp.tile([128, 512], F32, tag="ps2",
                                       name=f"ps2_{g}", bufs=2)
                nc.tensor.matmul(
                    out=ps2s[g][32 * q:32 * q + 16, :W],
                    lhsT=ws_sb[:, W2OFF + b * 16:W2OFF + (b + 1) * 16],
                    rhs=ths[b][:, :W],
                    start=True, stop=True,
                    tile_position=(0, 32 * q),
                )

            def emit_sigmoid(g):
                W = Wg[g]
                rows = 32 * gsizes[g]
                nc.scalar.activation(
                    out=y_sb[:rows, yoff[g]:yoff[g] + W],
                    in_=ps2s[g][:rows, :W],
                    func=mybir.ActivationFunctionType.Sigmoid,
                    bias=ws_sb[:rows, B2OFF + 2 * g:B2OFF + 2 * g + 2]
                    .bitcast(F32))
                # last group's y rides the Scalar ring right behind its own
                # sigmoid (no cross-engine handoff); earlier groups go on
                # Sync, idle once the x stream is issued.
                dma_eng = nc.scalar if g == NG - 1 else nc.sync
                dma_eng.dma_start(
                    out=y[:rows, yoff[g]:yoff[g] + W],
                    in_=y_sb[:rows, yoff[g]:yoff[g] + W])

            # software pipeline: MM2 trails MM1 by two banks so the
            # in-order Tensor queue never stalls on a fresh tanh; zero
            # matmuls slot into the early-bank gaps.
            # (no extra HAM warm-up matmuls: the PE never reaches 2.4GHz in
            # this environment, so re-zeroing passes are pure queue ballast)
            zero_mm(0)
            zero_mm(1)
            next_sig = 0
            for b in range(NB):
                emit_mm1(b)
                if b == 0:
                    zero_mm(2)
                elif b == 1:
                    zero_mm(3)
                elif b == 2:
                    zero_mm(4)
                    zero_mm(5)
                # MM2/sigmoid of older banks go ahead of this bank's tanh on
                # the in-order Scalar queue - their inputs are ready sooner
                if b >= 2:
                    emit_mm2(b - 2)
                    if (b - 2) == gend[next_sig]:
                        emit_sigmoid(next_sig)
                        next_sig += 1
                emit_tanh(b)
            for b in (NB - 2, NB - 1):
                emit_mm2(b)
                while next_sig < NG and gend[next_sig] <= b:
                    emit_sigmoid(next_sig)
                    next_sig += 1
            assert next_sig == NG, (next_sig, NG)

    _split_excess_waits(nc)
    return nc


# ------------------------------------------------------------------- driver
def _run(inputs, trace=False):
    bankW, gsizes, Wg, XC, YC, WS, in_maps, scat = _pack(**inputs)
    nc = _build(bankW, gsizes, Wg, XC, YC, WS)
    res = run_bass_kernel_spmd(
        nc, in_maps, core_ids=list(range(N_CORES)), trace=trace)
    out = np.zeros((B, 1), np.float32)
    for c in range(N_CORES):
        ydev = np.asarray(res.results[c]["y"], dtype=np.float32)  # (128, YC)
        for row, y0, n, samples in scat[c]:
            out[samples, 0] = ydev[row, y0:y0 + n]
    return out, res


def kernel(**inputs):
    out, _ = _run(inputs, trace=False)
    return out


# revision 50
# speedup vs baseline: 1.0312x; 1.0312x over previous
"""COMPASSNet MoE-routing kernel for 8 TRN2 NeuronCores (v3).

Problem: B=262144 samples of D=32 features with NaNs at 0/1/2 positions;
each of P=529 NaN patterns owns a tiny MLP (32 -> 4 -> 1, tanh/sigmoid).
y[b] = sigmoid(W2[p].tanh(x0[b] @ W1[p] + b1[p]) + b2[p]), p = pattern id.

Design: weights are the STATIONARY matmul operand (16-column LDWEIGHTS,
~13ns), X is the MOVING operand streaming at 1 column/cycle.

Host pack: samples sorted by pattern; each pattern split into k near-equal
chunks (k chosen globally so chunk count = 128*NB and sizes are uniform);
chunks dealt round-robin across 8 cores in size order.  Per core: NB banks
of 16 chunks (4 strips x 4 bands); bank b's column width W_b = its largest
chunk (pad columns are zero, ~1.5% total).  Banks group by 4 for the
sigmoid/output stage.

Device per bank b (W = bankW[b]):
  MM1 (x4):  lhsT = strip_j [128, 16] (4 patterns' W1, block diag),
             rhs = X_bj [128, W] (4 bands = 4 chunks' features),
             out = psum1[32j:32j+16, :W] - h on partitions, samples free.
  tanh:      ACT psum1 -> th (f16) with per-partition bias b1 (no carrier
             tricks; pattern 0 runs on device too).
  MM2:       lhsT = w2 block [128, 16], rhs = th,
             out = psum2[32q:32q+16, :W] (q = b%4) - the matmul does the
             cross-h reduction.
  sigmoid:   one ACT per 4-bank group on psum2 with per-partition bias b2
             (the matmul-unwritten psum regions it reads are only ever
             stale-finite because the zeroing matmuls ran first; their
             outputs are discarded at unpack).

Startup: a DVE memset + dummy tanh pull the ~1.3us ACT table load off the
critical path; six zeros x zeros matmuls zero every PSUM bank we use (so
partial-partition matmul writes never meet NaN garbage) and keep the PE
busy across the initial DMA wait.  Weights ride the Scalar HWDGE ring in
parallel with the x stream on Sync (DMA instruction issue costs ~0.7us
each, so few large transfers); MM2 trails MM1 by two banks so the in-order
Tensor queue never stalls on a fresh tanh.  Output order is unscrambled on
the host.
"""

import heapq

import numpy as np

import concourse.bass as bass
import concourse.tile as tile
from concourse import mybir
from concourse.bass_utils import run_bass_kernel_spmd


F32 = mybir.dt.float32
F16 = mybir.dt.float16
NP16 = np.float16

B = 262144
D = 32
P = 529
H = 4
N_CORES = 8
NB = 9          # banks per core (16 chunks each)


def _group_sizes(nb):
    # trailing groups shrink (…,3,2) so the tail-of-kernel sigmoid+y chain
    # covers only the narrowest banks
    gs = []
    while nb > 5:
        gs.append(4)
        nb -= 4
    if nb > 2:
        gs.append(nb - 2)
        nb = 2
    if nb:
        gs.append(nb)
    return gs


# ----------------------------------------------------------------- host pack
def _plan_chunks(counts):
    """Split patterns into 128*NB near-equal chunks.

    Returns chunks [(size, pattern, offset)] sorted by size desc and the
    per-group widths Wg (group g covers banks 4g..; W = its largest chunk).
    """
    target = 128 * NB
    h = []
    npieces = 0
    for p, n in enumerate(counts):
        n = int(n)
        if n == 0:
            continue
        k = (n + 511) // 512  # no piece may exceed 512 (PSUM bank width)
        heapq.heappush(h, (-((n + k - 1) // k), p, k))
        npieces += k
    assert npieces <= target, f"{npieces} chunks > {target} slots"
    while npieces < target:
        _, p, k = heapq.heappop(h)
        n = int(counts[p])
        k += 1
        heapq.heappush(h, (-((n + k - 1) // k), p, k))
        npieces += 1
    kmap = {p: k for _, p, k in h}
    chunks = []
    for p, n in enumerate(counts):
        n = int(n)
        if n == 0:
            continue
        k = kmap[p]
        base, rem = divmod(n, k)
        off = 0
        for i in range(k):
            sz = base + (1 if i < rem else 0)
            chunks.append((sz, p, off))
            off += sz
    chunks.sort(key=lambda c: -c[0])
    assert len(chunks) == target
    gsizes = _group_sizes(NB)
    # per-bank width = its largest chunk (chunks dealt 128 per bank row);
    # group width (sigmoid/y extent) = its first=widest bank
    bankW = [max(1, chunks[b * 128][0]) for b in range(NB)]
    Wg = []
    boff = 0
    for gs in gsizes:
        Wg.append(bankW[boff])
        boff += gs
    return chunks, bankW, Wg, gsizes


def _pack(x, pattern_ids, W1, b1, W2, b2):
    pid = np.asarray(pattern_ids).astype(np.int64).ravel()
    x0 = np.nan_to_num(np.asarray(x, dtype=np.float32))
    W1 = np.asarray(W1, dtype=np.float32)
    b1 = np.asarray(b1, dtype=np.float32)
    W2 = np.asarray(W2, dtype=np.float32)
    b2 = np.asarray(b2, dtype=np.float32)

    order = np.argsort(pid, kind="stable")
    counts = np.bincount(pid, minlength=P)
    starts = np.zeros(P + 1, np.int64)
    np.cumsum(counts, out=starts[1:])

    chunks, bankW, Wg, gsizes = _plan_chunks(counts)
    NG = len(gsizes)
    b2gq = []
    for g, gs in enumerate(gsizes):
        b2gq += [(g, q) for q in range(gs)]
    xoff = np.zeros(NB + 1, np.int64)
    for b in range(NB):
        xoff[b + 1] = xoff[b] + 4 * bankW[b]
    XC = int(xoff[NB])
    yoff = np.zeros(NG + 1, np.int64)
    for g in range(NG):
        yoff[g + 1] = yoff[g] + Wg[g]
    YC = int(yoff[NG])

    # ws layout (f16 cols): strips NB*64 | w2 NB*16 | b1(f32 as 2xf16) NB*2
    # | b2(f32 as 2xf16) NG*2
    W2OFF = NB * 64
    B1OFF = NB * 80
    B2OFF = B1OFF + NB * 2
    WS = B2OFF + NG * 2
    xs = [np.zeros((128, XC), NP16) for _ in range(N_CORES)]
    ws = [np.zeros((128, WS), NP16) for _ in range(N_CORES)]
    b1f = [np.zeros((128, NB), np.float32) for _ in range(N_CORES)]
    b2f = [np.zeros((128, NG), np.float32) for _ in range(N_CORES)]
    scat = [[] for _ in range(N_CORES)]  # (row, ycol0, n, sample_idx_array)

    hh = np.arange(H)
    for rank, (sz, p, off) in enumerate(chunks):
        if sz == 0:
            continue
        c = rank % N_CORES
        pos = rank // N_CORES
        b, k = divmod(pos, 16)
        j, t = divmod(k, 4)
        g, q = b2gq[b]
        W = bankW[b]
        samples = order[starts[p] + off: starts[p] + off + sz]
        xs[c][32 * t:32 * t + 32, xoff[b] + j * W: xoff[b] + j * W + sz] = \
            x0[samples].T
        ws[c][32 * t:32 * t + 32, (b * 4 + j) * 16 + 4 * t + hh] = W1[p]
        ws[c][32 * j + 4 * t + hh, W2OFF + b * 16 + 4 * j + t] = W2[p]
        b1f[c][32 * j + 4 * t + hh, b] = b1[p]
        b2f[c][32 * q + 4 * j + t, g] = b2[p]
        scat[c].append((32 * q + 4 * j + t, int(yoff[g]), sz, samples))

    for c in range(N_CORES):
        ws[c][:, B1OFF:B1OFF + NB * 2] = b1f[c].view(NP16)
        ws[c][:, B2OFF:B2OFF + NG * 2] = b2f[c].view(NP16)

    in_maps = [{"xs": xs[c], "ws": ws[c]} for c in range(N_CORES)]
    return bankW, gsizes, Wg, XC, YC, WS, in_maps, scat


# ------------------------------------------------------------- device build
def _split_excess_waits(nc, cap=1):
    """walrus rejects >1 sync wait per instruction; move extras onto
    same-engine NoOps placed immediately before the owner."""
    f = nc.m.functions[0]
    for bb in list(f.blocks):
        out, changed = [], False
        for inst in bb.instructions:
            si = inst.sync_info
            waits = list(si.on_wait) if si is not None else []
            if len(waits) > cap:
                for w in waits[:-cap]:
                    out.append(mybir.InstNoOp(
                        name=nc.get_next_instruction_name(),
                        sync_info=mybir.SyncInfo(on_wait=[w], on_update=[]),
                        bass_nofuse=True,
                        engine=inst.engine,
                    ))
                si.on_wait = waits[-cap:]
                changed = True
            out.append(inst)
        if changed:
            bb.instructions = out
    return nc


def _build(bankW, gsizes, Wg, XC, YC, WS):
    NG = len(gsizes)
    b2gq = []
    gend = []
    for g, gs in enumerate(gsizes):
        b2gq += [(g, q) for q in range(gs)]
        gend.append(len(b2gq) - 1)
    W2OFF = NB * 64
    B1OFF = NB * 80
    B2OFF = B1OFF + NB * 2
    nc = bass.Bass("TRN2", target_bir_lowering=False, debug=False)
    xs = nc.declare_dram_parameter("xs", [128, XC], F16, isOutput=False)
    wsd = nc.declare_dram_parameter("ws", [128, WS], F16, isOutput=False)
    # f32 y: the sigmoid ACT writes 4-byte output at full rate (2-byte
    # costs 1.7x); the extra DMA bytes are tiny
    y = nc.declare_dram_parameter("y", [128, YC], F32, isOutput=True)

    xoff = [0]
    for b in range(NB):
        xoff.append(xoff[-1] + 4 * bankW[b])
    yoff = [0]
    for g in range(NG):
        yoff.append(yoff[-1] + Wg[g])

    with tile.TileContext(nc) as tc:
        with (
            tc.tile_pool(name="consts", bufs=1) as consts,
            tc.tile_pool(name="rot", bufs=1) as rot,
            tc.tile_pool(name="ps", bufs=1, space="PSUM") as psp,
        ):
            # zero f16 source for the zeroing matmuls (DVE memset: Scalar
            # stays free for its weight DMA + ACT table load)
            wu = consts.tile([128, 512], F16)
            nc.vector.memset(wu, 0.0)

            # weights ride the Scalar HWDGE ring, in parallel with the x
            # stream on Sync; the dummy activation right after pulls the
            # ~1.3us ACT_TABLE_LOAD off the first real tanh's critical path
            # (tanh+sigmoid share a table set).  DMA instruction issue costs
            # ~0.7us each, so both streams use few, large transfers.
            ws_sb = consts.tile([128, WS], F16)
            nc.scalar.dma_start(out=ws_sb, in_=wsd[:, :])
            dummy = consts.tile([128, 1], F32)
            nc.scalar.activation(out=dummy, in_=wu[:, :1],
                                 func=mybir.ActivationFunctionType.Tanh)

            # x stream split across BOTH HWDGE rings (Sync + Scalar): total
            # HBM bandwidth is shared, but per-ring completion receipts
            # (~1.5-3us each) overlap instead of serializing, and the two
            # first chunks issue concurrently.  Single-bank chunks at the
            # head (earliest MM1 start) and tail (early tail semaphores).
            xts = {}
            # few large chunks: DMA completion receipts inflate with
            # descriptor-queue depth, so minimizing in-flight transfers gets
            # the tail banks' semaphores to fire sooner.
            # four large chunks: DMA completion receipts inflate with
            # descriptor-queue depth, so in-flight transfer count matters
            # more than per-bank granularity.
            chunk_plan = [
                (nc.sync, 0, 2),
                (nc.sync, 2, 4),
                (nc.sync, 4, 7),
                (nc.sync, 7, 9),
            ]
            assert chunk_plan[-1][2] == NB
            for ci, (eng, b0, b1e) in enumerate(chunk_plan):
                xt = consts.tile([128, xoff[b1e] - xoff[b0]], F16,
                                 tag=f"xt{ci}", name=f"xt{ci}")
                eng.dma_start(out=xt, in_=xs[:, xoff[b0]:xoff[b1e]])
                for b in range(b0, b1e):
                    xts[b] = (xt, xoff[b] - xoff[b0])

            y_sb = consts.tile([128, YC], F32)

            # PSUM zeroing (zeros x zeros): allocate all six tiles up front
            # so pool slots align bank b -> slot b%4, but emit the matmuls
            # spread through the first banks - they fill PE gaps while the
            # x stream ramps, keeping the HAM activity window busy (2.4GHz).
            zt = [psp.tile([128, 512], F32, tag="ps1", name=f"z1_{i}",
                           bufs=4) for i in range(4)]
            zt += [psp.tile([128, 512], F32, tag="ps2", name=f"z2_{i}",
                            bufs=2) for i in range(2)]

            def zero_mm(i):
                nc.tensor.matmul(out=zt[i], lhsT=wu[:, :128], rhs=wu,
                                 start=True, stop=True)

            ps1s, ths, ps2s = {}, {}, {}

            def emit_mm1(b):
                W = bankW[b]
                ps1 = psp.tile([128, 512], F32, tag="ps1", name=f"ps1_{b}",
                               bufs=4)
                ps1s[b] = ps1
                xt, xo = xts[b]
                for j in range(4):
                    nc.tensor.matmul(
                        out=ps1[32 * j:32 * j + 16, :W],
                        lhsT=ws_sb[:, (b * 4 + j) * 16:(b * 4 + j + 1) * 16],
                        rhs=xt[:, xo + j * W: xo + (j + 1) * W],
                        start=True, stop=True,
                        tile_position=(0, 32 * j),
                    )

            def emit_tanh(b):
                W = bankW[b]
                th = rot.tile([128, 512], F16, tag="th", name=f"th_{b}",
                              bufs=5)
                ths[b] = th
                nc.scalar.activation(
                    out=th[:, :W], in_=ps1s[b][:, :W],
                    func=mybir.ActivationFunctionType.Tanh,
                    bias=ws_sb[:, B1OFF + 2 * b:B1OFF + 2 * b + 2].bitcast(F32))

            def emit_mm2(b):
                W = bankW[b]
                g, q = b2gq[b]
                if q == 0:
                    ps2s[g] = psp.tile([128, 512], F32, tag="ps2",
                                       name=f"ps2_{g}", bufs=2)
                nc.tensor.matmul(
                    out=ps2s[g][32 * q:32 * q + 16, :W],
                    lhsT=ws_sb[:, W2OFF + b * 16:W2OFF + (b + 1) * 16],
                    rhs=ths[b][:, :W],
                    start=True, stop=True,
                    tile_position=(0, 32 * q),
                )

            def emit_sigmoid(g):
                W = Wg[g]
                rows = 32 * gsizes[g]
                nc.scalar.activation(
                    out=y_sb[:rows, yoff[g]:yoff[g] + W],
                    in_=ps2s[g][:rows, :W],
                    func=mybir.ActivationFunctionType.Sigmoid,
                    bias=ws_sb[:rows, B2OFF + 2 * g:B2OFF + 2 * g + 2]
                    .bitcast(F32))
                # last group's y rides the Scalar ring right behind its own
                # sigmoid (no cross-engine handoff); earlier groups go on
                # Sync, idle once the x stream is issued.
                dma_eng = nc.scalar if g == NG - 1 else nc.sync
                dma_eng.dma_start(
                    out=y[:rows, yoff[g]:yoff[g] + W],
                    in_=y_sb[:rows, yoff[g]:yoff[g] + W])

            # software pipeline: MM2 trails MM1 by two banks so the
            # in-order Tensor queue never stalls on a fresh tanh; zero
            # matmuls slot into the early-bank gaps.
            # (no extra HAM warm-up matmuls: the PE never reaches 2.4GHz in
            # this environment, so re-zeroing passes are pure queue ballast)
            zero_mm(0)
            zero_mm(1)
            next_sig = 0
            for b in range(NB):
                emit_mm1(b)
                if b == 0:
                    zero_mm(2)
                elif b == 1:
                    zero_mm(3)
                elif b == 2:
                    zero_mm(4)
                    zero_mm(5)
                # MM2/sigmoid of older banks go ahead of this bank's tanh on
                # the in-order Scalar queue - their inputs are ready sooner
                if b >= 2:
                    emit_mm2(b - 2)
                    if (b - 2) == gend[next_sig]:
                        emit_sigmoid(next_sig)
                        next_sig += 1
                emit_tanh(b)
            for b in (NB - 2, NB - 1):
                emit_mm2(b)
                while next_sig < NG and gend[next_sig] <= b:
                    emit_sigmoid(next_sig)
                    next_sig += 1
            assert next_sig == NG, (next_sig, NG)

    _split_excess_waits(nc)
    return nc


# ------------------------------------------------------------------- driver
def _run(inputs, trace=False):
    bankW, gsizes, Wg, XC, YC, WS, in_maps, scat = _pack(**inputs)
    nc = _build(bankW, gsizes, Wg, XC, YC, WS)
    res = run_bass_kernel_spmd(
        nc, in_maps, core_ids=list(range(N_CORES)), trace=trace)
    out = np.zeros((B, 1), np.float32)
    for c in range(N_CORES):
        ydev = np.asarray(res.results[c]["y"], dtype=np.float32)  # (128, YC)
        for row, y0, n, samples in scat[c]:
            out[samples, 0] = ydev[row, y0:y0 + n]
    return out, res


def kernel(**inputs):
    out, _ = _run(inputs, trace=False)
    return out


# revision 52
# speedup vs baseline: 1.0752x; 1.0427x over previous
"""COMPASSNet MoE-routing kernel for 8 TRN2 NeuronCores (v3).

Problem: B=262144 samples of D=32 features with NaNs at 0/1/2 positions;
each of P=529 NaN patterns owns a tiny MLP (32 -> 4 -> 1, tanh/sigmoid).
y[b] = sigmoid(W2[p].tanh(x0[b] @ W1[p] + b1[p]) + b2[p]), p = pattern id.

Design: weights are the STATIONARY matmul operand (16-column LDWEIGHTS,
~13ns), X is the MOVING operand streaming at 1 column/cycle.

Host pack: samples sorted by pattern; each pattern split into k near-equal
chunks (k chosen globally so chunk count = 128*NB and sizes are uniform);
chunks dealt round-robin across 8 cores in size order.  Per core: NB banks
of 16 chunks (4 strips x 4 bands); bank b's column width W_b = its largest
chunk (pad columns are zero, ~1.5% total).  Banks group by 4 for the
sigmoid/output stage.

Device per bank b (W = bankW[b]):
  MM1 (x4):  lhsT = strip_j [128, 16] (4 patterns' W1, block diag),
             rhs = X_bj [128, W] (4 bands = 4 chunks' features),
             out = psum1[32j:32j+16, :W] - h on partitions, samples free.
  tanh:      ACT psum1 -> th (f16) with per-partition bias b1 (no carrier
             tricks; pattern 0 runs on device too).
  MM2:       lhsT = w2 block [128, 16], rhs = th,
             out = psum2[32q:32q+16, :W] (q = b%4) - the matmul does the
             cross-h reduction.
  sigmoid:   one ACT per 4-bank group on psum2 with per-partition bias b2
             (the matmul-unwritten psum regions it reads are only ever
             stale-finite because the zeroing matmuls ran first; their
             outputs are discarded at unpack).

Startup: a DVE memset + dummy tanh pull the ~1.3us ACT table load off the
critical path; six zeros x zeros matmuls zero every PSUM bank we use (so
partial-partition matmul writes never meet NaN garbage) and keep the PE
busy across the initial DMA wait.  Weights ride the Scalar HWDGE ring in
parallel with the x stream on Sync (DMA instruction issue costs ~0.7us
each, so few large transfers); MM2 trails MM1 by two banks so the in-order
Tensor queue never stalls on a fresh tanh.  Output order is unscrambled on
the host.
"""

import heapq

import numpy as np

import concourse.bass as bass
import concourse.tile as tile
from concourse import mybir
from concourse.bass_utils import run_bass_kernel_spmd


F32 = mybir.dt.float32
F16 = mybir.dt.float16
NP16 = np.float16

B = 262144
D = 32
P = 529
H = 4
N_CORES = 8
NB = 9          # banks per core (16 chunks each)


def _group_sizes(nb):
    # trailing groups shrink (…,3,2) so the tail-of-kernel sigmoid+y chain
    # covers only the narrowest banks
    gs = []
    while nb > 5:
        gs.append(4)
        nb -= 4
    if nb > 2:
        gs.append(nb - 2)
        nb = 2
    if nb:
        gs.append(nb)
    return gs


# ----------------------------------------------------------------- host pack
def _plan_chunks(counts):
    """Split patterns into 128*NB near-equal chunks.

    Returns chunks [(size, pattern, offset)] sorted by size desc and the
    per-group widths Wg (group g covers banks 4g..; W = its largest chunk).
    """
    target = 128 * NB
    h = []
    npieces = 0
    for p, n in enumerate(counts):
        n = int(n)
        if n == 0:
            continue
        k = (n + 511) // 512  # no piece may exceed 512 (PSUM bank width)
        heapq.heappush(h, (-((n + k - 1) // k), p, k))
        npieces += k
    assert npieces <= target, f"{npieces} chunks > {target} slots"
    while npieces < target:
        _, p, k = heapq.heappop(h)
        n = int(counts[p])
        k += 1
        heapq.heappush(h, (-((n + k - 1) // k), p, k))
        npieces += 1
    kmap = {p: k for _, p, k in h}
    chunks = []
    for p, n in enumerate(counts):
        n = int(n)
        if n == 0:
            continue
        k = kmap[p]
        base, rem = divmod(n, k)
        off = 0
        for i in range(k):
            sz = base + (1 if i < rem else 0)
            chunks.append((sz, p, off))
            off += sz
    chunks.sort(key=lambda c: -c[0])
    assert len(chunks) == target
    gsizes = _group_sizes(NB)
    # per-bank width = its largest chunk (chunks dealt 128 per bank row);
    # group width (sigmoid/y extent) = its first=widest bank
    bankW = [max(1, chunks[b * 128][0]) for b in range(NB)]
    Wg = []
    boff = 0
    for gs in gsizes:
        Wg.append(bankW[boff])
        boff += gs
    return chunks, bankW, Wg, gsizes


def _pack(x, pattern_ids, W1, b1, W2, b2):
    pid = np.asarray(pattern_ids).astype(np.int64).ravel()
    x0 = np.nan_to_num(np.asarray(x, dtype=np.float32))
    W1 = np.asarray(W1, dtype=np.float32)
    b1 = np.asarray(b1, dtype=np.float32)
    W2 = np.asarray(W2, dtype=np.float32)
    b2 = np.asarray(b2, dtype=np.float32)

    order = np.argsort(pid, kind="stable")
    counts = np.bincount(pid, minlength=P)
    starts = np.zeros(P + 1, np.int64)
    np.cumsum(counts, out=starts[1:])

    chunks, bankW, Wg, gsizes = _plan_chunks(counts)
    NG = len(gsizes)
    b2gq = []
    for g, gs in enumerate(gsizes):
        b2gq += [(g, q) for q in range(gs)]
    xoff = np.zeros(NB + 1, np.int64)
    for b in range(NB):
        xoff[b + 1] = xoff[b] + 4 * bankW[b]
    XC = int(xoff[NB])
    yoff = np.zeros(NG + 1, np.int64)
    for g in range(NG):
        yoff[g + 1] = yoff[g] + Wg[g]
    YC = int(yoff[NG])

    # ws layout (f16 cols): strips NB*64 | w2 NB*16 | b1(f32 as 2xf16) NB*2
    # | b2(f32 as 2xf16) NG*2
    W2OFF = NB * 64
    B1OFF = NB * 80
    B2OFF = B1OFF + NB * 2
    WS = B2OFF + NG * 2
    xs = [np.zeros((128, XC), NP16) for _ in range(N_CORES)]
    ws = [np.zeros((128, WS), NP16) for _ in range(N_CORES)]
    b1f = [np.zeros((128, NB), np.float32) for _ in range(N_CORES)]
    b2f = [np.zeros((128, NG), np.float32) for _ in range(N_CORES)]
    scat = [[] for _ in range(N_CORES)]  # (row, ycol0, n, sample_idx_array)

    hh = np.arange(H)
    for rank, (sz, p, off) in enumerate(chunks):
        if sz == 0:
            continue
        c = rank % N_CORES
        pos = rank // N_CORES
        b, k = divmod(pos, 16)
        j, t = divmod(k, 4)
        g, q = b2gq[b]
        W = bankW[b]
        samples = order[starts[p] + off: starts[p] + off + sz]
        xs[c][32 * t:32 * t + 32, xoff[b] + j * W: xoff[b] + j * W + sz] = \
            x0[samples].T
        ws[c][32 * t:32 * t + 32, (b * 4 + j) * 16 + 4 * t + hh] = W1[p]
        ws[c][32 * j + 4 * t + hh, W2OFF + b * 16 + 4 * j + t] = W2[p]
        b1f[c][32 * j + 4 * t + hh, b] = b1[p]
        b2f[c][32 * q + 4 * j + t, g] = b2[p]
        scat[c].append((32 * q + 4 * j + t, int(yoff[g]), sz, samples))

    for c in range(N_CORES):
        ws[c][:, B1OFF:B1OFF + NB * 2] = b1f[c].view(NP16)
        ws[c][:, B2OFF:B2OFF + NG * 2] = b2f[c].view(NP16)

    in_maps = [{"xs": xs[c], "ws": ws[c]} for c in range(N_CORES)]
    return bankW, gsizes, Wg, XC, YC, WS, in_maps, scat


# ------------------------------------------------------------- device build
def _split_excess_waits(nc, cap=1):
    """walrus rejects >1 sync wait per instruction; move extras onto
    same-engine NoOps placed immediately before the owner."""
    f = nc.m.functions[0]
    for bb in list(f.blocks):
        out, changed = [], False
        for inst in bb.instructions:
            si = inst.sync_info
            waits = list(si.on_wait) if si is not None else []
            if len(waits) > cap:
                for w in waits[:-cap]:
                    out.append(mybir.InstNoOp(
                        name=nc.get_next_instruction_name(),
                        sync_info=mybir.SyncInfo(on_wait=[w], on_update=[]),
                        bass_nofuse=True,
                        engine=inst.engine,
                    ))
                si.on_wait = waits[-cap:]
                changed = True
            out.append(inst)
        if changed:
            bb.instructions = out
    return nc


def _build(bankW, gsizes, Wg, XC, YC, WS):
    NG = len(gsizes)
    b2gq = []
    gend = []
    for g, gs in enumerate(gsizes):
        b2gq += [(g, q) for q in range(gs)]
        gend.append(len(b2gq) - 1)
    W2OFF = NB * 64
    B1OFF = NB * 80
    B2OFF = B1OFF + NB * 2
    nc = bass.Bass("TRN2", target_bir_lowering=False, debug=False)
    xs = nc.declare_dram_parameter("xs", [128, XC], F16, isOutput=False)
    wsd = nc.declare_dram_parameter("ws", [128, WS], F16, isOutput=False)
    # f16 y: measured faster than f32 despite the sigmoid ACT's 1.7x
    # 2-byte write penalty - halving the output DMA bytes wins
    y = nc.declare_dram_parameter("y", [128, YC], F16, isOutput=True)

    xoff = [0]
    for b in range(NB):
        xoff.append(xoff[-1] + 4 * bankW[b])
    yoff = [0]
    for g in range(NG):
        yoff.append(yoff[-1] + Wg[g])

    with tile.TileContext(nc) as tc:
        with (
            tc.tile_pool(name="consts", bufs=1) as consts,
            tc.tile_pool(name="rot", bufs=1) as rot,
            tc.tile_pool(name="ps", bufs=1, space="PSUM") as psp,
        ):
            # zero f16 source for the zeroing matmuls (DVE memset: Scalar
            # stays free for its weight DMA + ACT table load)
            wu = consts.tile([128, 512], F16)
            nc.vector.memset(wu, 0.0)

            # weights ride the Scalar HWDGE ring, in parallel with the x
            # stream on Sync; the dummy activation right after pulls the
            # ~1.3us ACT_TABLE_LOAD off the first real tanh's critical path
            # (tanh+sigmoid share a table set).  DMA instruction issue costs
            # ~0.7us each, so both streams use few, large transfers.
            ws_sb = consts.tile([128, WS], F16)
            nc.scalar.dma_start(out=ws_sb, in_=wsd[:, :])
            dummy = consts.tile([128, 1], F32)
            nc.scalar.activation(out=dummy, in_=wu[:, :1],
                                 func=mybir.ActivationFunctionType.Tanh)

            # x stream split across BOTH HWDGE rings (Sync + Scalar): total
            # HBM bandwidth is shared, but per-ring completion receipts
            # (~1.5-3us each) overlap instead of serializing, and the two
            # first chunks issue concurrently.  Single-bank chunks at the
            # head (earliest MM1 start) and tail (early tail semaphores).
            xts = {}
            # few large chunks: DMA completion receipts inflate with
            # descriptor-queue depth, so minimizing in-flight transfers gets
            # the tail banks' semaphores to fire sooner.
            # four large chunks: DMA completion receipts inflate with
            # descriptor-queue depth, so in-flight transfer count matters
            # more than per-bank granularity.
            chunk_plan = [
                (nc.sync, 0, 2),
                (nc.sync, 2, 4),
                (nc.sync, 4, 7),
                (nc.sync, 7, 9),
            ]
            assert chunk_plan[-1][2] == NB
            for ci, (eng, b0, b1e) in enumerate(chunk_plan):
                xt = consts.tile([128, xoff[b1e] - xoff[b0]], F16,
                                 tag=f"xt{ci}", name=f"xt{ci}")
                eng.dma_start(out=xt, in_=xs[:, xoff[b0]:xoff[b1e]])
                for b in range(b0, b1e):
                    xts[b] = (xt, xoff[b] - xoff[b0])

            y_sb = consts.tile([128, YC], F16)

            # PSUM zeroing (zeros x zeros): allocate all six tiles up front
            # so pool slots align bank b -> slot b%4, but emit the matmuls
            # spread through the first banks - they fill PE gaps while the
            # x stream ramps, keeping the HAM activity window busy (2.4GHz).
            zt = [psp.tile([128, 512], F32, tag="ps1", name=f"z1_{i}",
                           bufs=4) for i in range(4)]
            zt += [psp.tile([128, 512], F32, tag="ps2", name=f"z2_{i}",
                            bufs=2) for i in range(2)]

            def zero_mm(i):
                nc.tensor.matmul(out=zt[i], lhsT=wu[:, :128], rhs=wu,
                                 start=True, stop=True)

            ps1s, ths, ps2s = {}, {}, {}

            def emit_mm1(b):
                W = bankW[b]
                ps1 = psp.tile([128, 512], F32, tag="ps1", name=f"ps1_{b}",
                               bufs=4)
                ps1s[b] = ps1
                xt, xo = xts[b]
                for j in range(4):
                    nc.tensor.matmul(
                        out=ps1[32 * j:32 * j + 16, :W],
                        lhsT=ws_sb[:, (b * 4 + j) * 16:(b * 4 + j + 1) * 16],
                        rhs=xt[:, xo + j * W: xo + (j + 1) * W],
                        start=True, stop=True,
                        tile_position=(0, 32 * j),
                    )

            def emit_tanh(b):
                W = bankW[b]
                th = rot.tile([128, 512], F16, tag="th", name=f"th_{b}",
                              bufs=5)
                ths[b] = th
                nc.scalar.activation(
                    out=th[:, :W], in_=ps1s[b][:, :W],
                    func=mybir.ActivationFunctionType.Tanh,
                    bias=ws_sb[:, B1OFF + 2 * b:B1OFF + 2 * b + 2].bitcast(F32))

            def emit_mm2(b):
                W = bankW[b]
                g, q = b2gq[b]
                if q == 0:
                    ps2s[g] = psp.tile([128, 512], F32, tag="ps2",
                                       name=f"ps2_{g}", bufs=2)
                nc.tensor.matmul(
                    out=ps2s[g][32 * q:32 * q + 16, :W],
                    lhsT=ws_sb[:, W2OFF + b * 16:W2OFF + (b + 1) * 16],
                    rhs=ths[b][:, :W],
                    start=True, stop=True,
                    tile_position=(0, 32 * q),
                )

            def emit_sigmoid(g):
                W = Wg[g]
                rows = 32 * gsizes[g]
                nc.scalar.activation(
                    out=y_sb[:rows, yoff[g]:yoff[g] + W],
                    in_=ps2s[g][:rows, :W],
                    func=mybir.ActivationFunctionType.Sigmoid,
                    bias=ws_sb[:rows, B2OFF + 2 * g:B2OFF + 2 * g + 2]
                    .bitcast(F32))
                # last group's y rides the Scalar ring right behind its own
                # sigmoid (no cross-engine handoff); earlier groups go on
                # Sync, idle once the x stream is issued.
                dma_eng = nc.scalar if g == NG - 1 else nc.sync
                dma_eng.dma_start(
                    out=y[:rows, yoff[g]:yoff[g] + W],
                    in_=y_sb[:rows, yoff[g]:yoff[g] + W])

            # software pipeline: MM2 trails MM1 by two banks so the
            # in-order Tensor queue never stalls on a fresh tanh; zero
            # matmuls slot into the early-bank gaps.
            # (no extra HAM warm-up matmuls: the PE never reaches 2.4GHz in
            # this environment, so re-zeroing passes are pure queue ballast)
            zero_mm(0)
            zero_mm(1)
            next_sig = 0
            for b in range(NB):
                emit_mm1(b)
                if b == 0:
                    zero_mm(2)
                elif b == 1:
                    zero_mm(3)
                elif b == 2:
                    zero_mm(4)
                    zero_mm(5)
                # MM2/sigmoid of older banks go ahead of this bank's tanh on
                # the in-order Scalar queue - their inputs are ready sooner
                if b >= 2:
                    emit_mm2(b - 2)
                    if (b - 2) == gend[next_sig]:
                        emit_sigmoid(next_sig)
                        next_sig += 1
                emit_tanh(b)
            for b in (NB - 2, NB - 1):
                emit_mm2(b)
                while next_sig < NG and gend[next_sig] <= b:
                    emit_sigmoid(next_sig)
                    next_sig += 1
            assert next_sig == NG, (next_sig, NG)

    _split_excess_waits(nc)
    return nc


# ------------------------------------------------------------------- driver
def _run(inputs, trace=False):
    bankW, gsizes, Wg, XC, YC, WS, in_maps, scat = _pack(**inputs)
    nc = _build(bankW, gsizes, Wg, XC, YC, WS)
    res = run_bass_kernel_spmd(
        nc, in_maps, core_ids=list(range(N_CORES)), trace=trace)
    out = np.zeros((B, 1), np.float32)
    for c in range(N_CORES):
        ydev = np.asarray(res.results[c]["y"], dtype=np.float32)  # (128, YC)
        for row, y0, n, samples in scat[c]:
            out[samples, 0] = ydev[row, y0:y0 + n]
    return out, res


def kernel(**inputs):
    out, _ = _run(inputs, trace=False)
    return out


# revision 53
# speedup vs baseline: 1.1246x; 1.0460x over previous
"""COMPASSNet MoE-routing kernel for 8 TRN2 NeuronCores (v3).

Problem: B=262144 samples of D=32 features with NaNs at 0/1/2 positions;
each of P=529 NaN patterns owns a tiny MLP (32 -> 4 -> 1, tanh/sigmoid).
y[b] = sigmoid(W2[p].tanh(x0[b] @ W1[p] + b1[p]) + b2[p]), p = pattern id.

Design: weights are the STATIONARY matmul operand (16-column LDWEIGHTS,
~13ns), X is the MOVING operand streaming at 1 column/cycle.

Host pack: samples sorted by pattern; each pattern split into k near-equal
chunks (k chosen globally so chunk count = 128*NB and sizes are uniform);
chunks dealt round-robin across 8 cores in size order.  Per core: NB banks
of 16 chunks (4 strips x 4 bands); bank b's column width W_b = its largest
chunk (pad columns are zero, ~1.5% total).  Banks group by 4 for the
sigmoid/output stage.

Device per bank b (W = bankW[b]):
  MM1 (x4):  lhsT = strip_j [128, 16] (4 patterns' W1, block diag),
             rhs = X_bj [128, W] (4 bands = 4 chunks' features),
             out = psum1[32j:32j+16, :W] - h on partitions, samples free.
  tanh:      ACT psum1 -> th (f16) with per-partition bias b1 (no carrier
             tricks; pattern 0 runs on device too).
  MM2:       lhsT = w2 block [128, 16], rhs = th,
             out = psum2[32q:32q+16, :W] (q = b%4) - the matmul does the
             cross-h reduction.
  sigmoid:   one ACT per 4-bank group on psum2 with per-partition bias b2
             (the matmul-unwritten psum regions it reads are only ever
             stale-finite because the zeroing matmuls ran first; their
             outputs are discarded at unpack).

Startup: a DVE memset + dummy tanh pull the ~1.3us ACT table load off the
critical path; six zeros x zeros matmuls zero every PSUM bank we use (so
partial-partition matmul writes never meet NaN garbage) and keep the PE
busy across the initial DMA wait.  Weights ride the Scalar HWDGE ring in
parallel with the x stream on Sync (DMA instruction issue costs ~0.7us
each, so few large transfers); MM2 trails MM1 by two banks so the in-order
Tensor queue never stalls on a fresh tanh.  Output order is unscrambled on
the host.
"""

import heapq

import numpy as np

import concourse.bass as bass
import concourse.tile as tile
from concourse import mybir
from concourse.bass_utils import run_bass_kernel_spmd


F32 = mybir.dt.float32
F16 = mybir.dt.float16
NP16 = np.float16

B = 262144
D = 32
P = 529
H = 4
N_CORES = 8
NB = 9          # banks per core (16 chunks each)


def _group_sizes(nb):
    # trailing groups shrink (…,3,2) so the tail-of-kernel sigmoid+y chain
    # covers only the narrowest banks
    gs = []
    while nb > 5:
        gs.append(4)
        nb -= 4
    if nb > 2:
        gs.append(nb - 2)
        nb = 2
    if nb:
        gs.append(nb)
    return gs


# ----------------------------------------------------------------- host pack
def _plan_chunks(counts):
    """Split patterns into 128*NB near-equal chunks.

    Returns chunks [(size, pattern, offset)] sorted by size desc and the
    per-group widths Wg (group g covers banks 4g..; W = its largest chunk).
    """
    target = 128 * NB
    h = []
    npieces = 0
    for p, n in enumerate(counts):
        n = int(n)
        if n == 0:
            continue
        k = (n + 511) // 512  # no piece may exceed 512 (PSUM bank width)
        heapq.heappush(h, (-((n + k - 1) // k), p, k))
        npieces += k
    assert npieces <= target, f"{npieces} chunks > {target} slots"
    while npieces < target:
        _, p, k = heapq.heappop(h)
        n = int(counts[p])
        k += 1
        heapq.heappush(h, (-((n + k - 1) // k), p, k))
        npieces += 1
    kmap = {p: k for _, p, k in h}
    chunks = []
    for p, n in enumerate(counts):
        n = int(n)
        if n == 0:
            continue
        k = kmap[p]
        base, rem = divmod(n, k)
        off = 0
        for i in range(k):
            sz = base + (1 if i < rem else 0)
            chunks.append((sz, p, off))
            off += sz
    chunks.sort(key=lambda c: -c[0])
    assert len(chunks) == target
    gsizes = _group_sizes(NB)
    # per-bank width = its largest chunk (chunks dealt 128 per bank row);
    # group width (sigmoid/y extent) = its first=widest bank
    bankW = [max(1, chunks[b * 128][0]) for b in range(NB)]
    Wg = []
    boff = 0
    for gs in gsizes:
        Wg.append(bankW[boff])
        boff += gs
    return chunks, bankW, Wg, gsizes


def _pack(x, pattern_ids, W1, b1, W2, b2):
    pid = np.asarray(pattern_ids).astype(np.int64).ravel()
    x0 = np.nan_to_num(np.asarray(x, dtype=np.float32))
    W1 = np.asarray(W1, dtype=np.float32)
    b1 = np.asarray(b1, dtype=np.float32)
    W2 = np.asarray(W2, dtype=np.float32)
    b2 = np.asarray(b2, dtype=np.float32)

    order = np.argsort(pid, kind="stable")
    counts = np.bincount(pid, minlength=P)
    starts = np.zeros(P + 1, np.int64)
    np.cumsum(counts, out=starts[1:])

    chunks, bankW, Wg, gsizes = _plan_chunks(counts)
    NG = len(gsizes)
    b2gq = []
    for g, gs in enumerate(gsizes):
        b2gq += [(g, q) for q in range(gs)]
    xoff = np.zeros(NB + 1, np.int64)
    for b in range(NB):
        xoff[b + 1] = xoff[b] + 4 * bankW[b]
    XC = int(xoff[NB])
    yoff = np.zeros(NG + 1, np.int64)
    for g in range(NG):
        yoff[g + 1] = yoff[g] + Wg[g]
    YC = int(yoff[NG])

    # ws layout (f16 cols): strips NB*64 | w2 NB*16 | b1(f32 as 2xf16) NB*2
    # | b2(f32 as 2xf16) NG*2
    W2OFF = NB * 64
    B1OFF = NB * 80
    B2OFF = B1OFF + NB * 2
    WS = B2OFF + NG * 2
    xs = [np.zeros((128, XC), NP16) for _ in range(N_CORES)]
    ws = [np.zeros((128, WS), NP16) for _ in range(N_CORES)]
    b1f = [np.zeros((128, NB), np.float32) for _ in range(N_CORES)]
    b2f = [np.zeros((128, NG), np.float32) for _ in range(N_CORES)]
    scat = [[] for _ in range(N_CORES)]  # (row, ycol0, n, sample_idx_array)

    hh = np.arange(H)
    for rank, (sz, p, off) in enumerate(chunks):
        if sz == 0:
            continue
        c = rank % N_CORES
        pos = rank // N_CORES
        b, k = divmod(pos, 16)
        j, t = divmod(k, 4)
        g, q = b2gq[b]
        W = bankW[b]
        samples = order[starts[p] + off: starts[p] + off + sz]
        xs[c][32 * t:32 * t + 32, xoff[b] + j * W: xoff[b] + j * W + sz] = \
            x0[samples].T
        ws[c][32 * t:32 * t + 32, (b * 4 + j) * 16 + 4 * t + hh] = W1[p]
        ws[c][32 * j + 4 * t + hh, W2OFF + b * 16 + 4 * j + t] = W2[p]
        b1f[c][32 * j + 4 * t + hh, b] = b1[p]
        b2f[c][32 * q + 4 * j + t, g] = b2[p]
        scat[c].append((32 * q + 4 * j + t, int(yoff[g]), sz, samples))

    for c in range(N_CORES):
        ws[c][:, B1OFF:B1OFF + NB * 2] = b1f[c].view(NP16)
        ws[c][:, B2OFF:B2OFF + NG * 2] = b2f[c].view(NP16)

    in_maps = [{"xs": xs[c], "ws": ws[c]} for c in range(N_CORES)]
    return bankW, gsizes, Wg, XC, YC, WS, in_maps, scat


# ------------------------------------------------------------- device build
def _split_excess_waits(nc, cap=1):
    """walrus rejects >1 sync wait per instruction; move extras onto
    same-engine NoOps placed immediately before the owner."""
    f = nc.m.functions[0]
    for bb in list(f.blocks):
        out, changed = [], False
        for inst in bb.instructions:
            si = inst.sync_info
            waits = list(si.on_wait) if si is not None else []
            if len(waits) > cap:
                for w in waits[:-cap]:
                    out.append(mybir.InstNoOp(
                        name=nc.get_next_instruction_name(),
                        sync_info=mybir.SyncInfo(on_wait=[w], on_update=[]),
                        bass_nofuse=True,
                        engine=inst.engine,
                    ))
                si.on_wait = waits[-cap:]
                changed = True
            out.append(inst)
        if changed:
            bb.instructions = out
    return nc


def _build(bankW, gsizes, Wg, XC, YC, WS):
    NG = len(gsizes)
    b2gq = []
    gend = []
    for g, gs in enumerate(gsizes):
        b2gq += [(g, q) for q in range(gs)]
        gend.append(len(b2gq) - 1)
    W2OFF = NB * 64
    B1OFF = NB * 80
    B2OFF = B1OFF + NB * 2
    nc = bass.Bass("TRN2", target_bir_lowering=False, debug=False)
    xs = nc.declare_dram_parameter("xs", [128, XC], F16, isOutput=False)
    wsd = nc.declare_dram_parameter("ws", [128, WS], F16, isOutput=False)
    # f16 y: measured faster than f32 despite the sigmoid ACT's 1.7x
    # 2-byte write penalty - halving the output DMA bytes wins
    y = nc.declare_dram_parameter("y", [128, YC], F16, isOutput=True)

    xoff = [0]
    for b in range(NB):
        xoff.append(xoff[-1] + 4 * bankW[b])
    yoff = [0]
    for g in range(NG):
        yoff.append(yoff[-1] + Wg[g])

    with tile.TileContext(nc) as tc:
        with (
            tc.tile_pool(name="consts", bufs=1) as consts,
            tc.tile_pool(name="rot", bufs=1) as rot,
            tc.tile_pool(name="ps", bufs=1, space="PSUM") as psp,
        ):
            # zero f16 source for the zeroing matmuls (DVE memset: Scalar
            # stays free for its weight DMA + ACT table load)
            wu = consts.tile([128, 512], F16)
            nc.vector.memset(wu, 0.0)

            # weights ride the Scalar HWDGE ring, in parallel with the x
            # stream on Sync; the dummy activation right after pulls the
            # ~1.3us ACT_TABLE_LOAD off the first real tanh's critical path
            # (tanh+sigmoid share a table set).  DMA instruction issue costs
            # ~0.7us each, so both streams use few, large transfers.
            ws_sb = consts.tile([128, WS], F16)
            nc.scalar.dma_start(out=ws_sb, in_=wsd[:, :])
            dummy = consts.tile([128, 1], F32)
            nc.scalar.activation(out=dummy, in_=wu[:, :1],
                                 func=mybir.ActivationFunctionType.Tanh)

            # x stream split across BOTH HWDGE rings (Sync + Scalar): total
            # HBM bandwidth is shared, but per-ring completion receipts
            # (~1.5-3us each) overlap instead of serializing, and the two
            # first chunks issue concurrently.  Single-bank chunks at the
            # head (earliest MM1 start) and tail (early tail semaphores).
            xts = {}
            # few large chunks: DMA completion receipts inflate with
            # descriptor-queue depth, so minimizing in-flight transfers gets
            # the tail banks' semaphores to fire sooner.
            # four large chunks: DMA completion receipts inflate with
            # descriptor-queue depth, so in-flight transfer count matters
            # more than per-bank granularity.
            chunk_plan = [
                (nc.sync, 0, 2),
                (nc.sync, 2, 4),
                (nc.sync, 4, 7),
                (nc.sync, 7, 9),
            ]
            assert chunk_plan[-1][2] == NB
            for ci, (eng, b0, b1e) in enumerate(chunk_plan):
                xt = consts.tile([128, xoff[b1e] - xoff[b0]], F16,
                                 tag=f"xt{ci}", name=f"xt{ci}")
                eng.dma_start(out=xt, in_=xs[:, xoff[b0]:xoff[b1e]])
                for b in range(b0, b1e):
                    xts[b] = (xt, xoff[b] - xoff[b0])

            y_sb = consts.tile([128, YC], F16)

            # PSUM zeroing (zeros x zeros): allocate all six tiles up front
            # so pool slots align bank b -> slot b%4, but emit the matmuls
            # spread through the first banks - they fill PE gaps while the
            # x stream ramps, keeping the HAM activity window busy (2.4GHz).
            zt = [psp.tile([128, 512], F32, tag="ps1", name=f"z1_{i}",
                           bufs=4) for i in range(4)]
            zt += [psp.tile([128, 512], F32, tag="ps2", name=f"z2_{i}",
                            bufs=2) for i in range(2)]

            # zero only the columns ever read (tanh/sigmoid extents are
            # bounded by the widest bank) - halves each zero-matmul's
            # streaming time on the Tensor queue
            Wz = max(bankW)

            def zero_mm(i):
                nc.tensor.matmul(out=zt[i][:, :Wz], lhsT=wu[:, :128],
                                 rhs=wu[:, :Wz], start=True, stop=True)

            ps1s, ths, ps2s = {}, {}, {}

            def emit_mm1(b):
                W = bankW[b]
                ps1 = psp.tile([128, 512], F32, tag="ps1", name=f"ps1_{b}",
                               bufs=4)
                ps1s[b] = ps1
                xt, xo = xts[b]
                for j in range(4):
                    nc.tensor.matmul(
                        out=ps1[32 * j:32 * j + 16, :W],
                        lhsT=ws_sb[:, (b * 4 + j) * 16:(b * 4 + j + 1) * 16],
                        rhs=xt[:, xo + j * W: xo + (j + 1) * W],
                        start=True, stop=True,
                        tile_position=(0, 32 * j),
                    )

            def emit_tanh(b):
                W = bankW[b]
                th = rot.tile([128, 512], F16, tag="th", name=f"th_{b}",
                              bufs=5)
                ths[b] = th
                nc.scalar.activation(
                    out=th[:, :W], in_=ps1s[b][:, :W],
                    func=mybir.ActivationFunctionType.Tanh,
                    bias=ws_sb[:, B1OFF + 2 * b:B1OFF + 2 * b + 2].bitcast(F32))

            def emit_mm2(b):
                W = bankW[b]
                g, q = b2gq[b]
                if q == 0:
                    ps2s[g] = psp.tile([128, 512], F32, tag="ps2",
                                       name=f"ps2_{g}", bufs=2)
                nc.tensor.matmul(
                    out=ps2s[g][32 * q:32 * q + 16, :W],
                    lhsT=ws_sb[:, W2OFF + b * 16:W2OFF + (b + 1) * 16],
                    rhs=ths[b][:, :W],
                    start=True, stop=True,
                    tile_position=(0, 32 * q),
                )

            def emit_sigmoid(g):
                W = Wg[g]
                rows = 32 * gsizes[g]
                nc.scalar.activation(
                    out=y_sb[:rows, yoff[g]:yoff[g] + W],
                    in_=ps2s[g][:rows, :W],
                    func=mybir.ActivationFunctionType.Sigmoid,
                    bias=ws_sb[:rows, B2OFF + 2 * g:B2OFF + 2 * g + 2]
                    .bitcast(F32))
                # last group's y rides the Scalar ring right behind its own
                # sigmoid (no cross-engine handoff); earlier groups go on
                # Sync, idle once the x stream is issued.
                dma_eng = nc.scalar if g == NG - 1 else nc.sync
                dma_eng.dma_start(
                    out=y[:rows, yoff[g]:yoff[g] + W],
                    in_=y_sb[:rows, yoff[g]:yoff[g] + W])

            # software pipeline: MM2 trails MM1 by two banks so the
            # in-order Tensor queue never stalls on a fresh tanh; zero
            # matmuls slot into the early-bank gaps.
            # (no extra HAM warm-up matmuls: the PE never reaches 2.4GHz in
            # this environment, so re-zeroing passes are pure queue ballast)
            zero_mm(0)
            zero_mm(1)
            next_sig = 0
            for b in range(NB):
                emit_mm1(b)
                if b == 0:
                    zero_mm(2)
                elif b == 1:
                    zero_mm(3)
                elif b == 2:
                    zero_mm(4)
                    zero_mm(5)
                # MM2/sigmoid of older banks go ahead of this bank's tanh on
                # the in-order Scalar queue - their inputs are ready sooner
                if b >= 2:
                    emit_mm2(b - 2)
                    if (b - 2) == gend[next_sig]:
                        emit_sigmoid(next_sig)
                        next_sig += 1
                emit_tanh(b)
            for b in (NB - 2, NB - 1):
                emit_mm2(b)
                while next_sig < NG and gend[next_sig] <= b:
                    emit_sigmoid(next_sig)
                    next_sig += 1
            assert next_sig == NG, (next_sig, NG)

    _split_excess_waits(nc)
    return nc


# ------------------------------------------------------------------- driver
def _run(inputs, trace=False):
    bankW, gsizes, Wg, XC, YC, WS, in_maps, scat = _pack(**inputs)
    nc = _build(bankW, gsizes, Wg, XC, YC, WS)
    res = run_bass_kernel_spmd(
        nc, in_maps, core_ids=list(range(N_CORES)), trace=trace)
    out = np.zeros((B, 1), np.float32)
    for c in range(N_CORES):
        ydev = np.asarray(res.results[c]["y"], dtype=np.float32)  # (128, YC)
        for row, y0, n, samples in scat[c]:
            out[samples, 0] = ydev[row, y0:y0 + n]
    return out, res


def kernel(**inputs):
    out, _ = _run(inputs, trace=False)
    return out
